# revision 8
# baseline (speedup 1.0000x reference)
"""Compositional attention kernel for Trainium2, 8-core SPMD.

Sharding: core c handles batch b = c // 4 and search-heads {2*(c%4), 2*(c%4)+1}
end-to-end (tensor-parallel over the S=8 search heads x data-parallel over
B=2).  Each core computes a partial y = out_heads @ w_out[head_rows] for its
batch; the host sums the 4 partials per batch.

Schedule: one globally interleaved emission keeps the PE saturated (and at
full pstate clock) while the Scalar engine's exp stream - the 67us floor -
runs continuously:
  A: proj q,k (both heads packed [128, n])
  B: scores_h0 units (st psum -> exp on ACT) interleaved with proj rq, rv
     (+ PE transposes), then Z_h0/AV_h0 chains for the first half of i
  C: scores_h1 units interleaved with the rest of Z_h0/AV_h0, u_h0
     transposes, rqW_h0, stage-2 h0 (DVE)
  D: Z_h1 + AV_h1 chains interleaved, uf_h0 transposes
  E: u_h1 transposes, rqW_h1, stage-2 h1, uf_h1 + out-proj per i-tile

Z = sum_j exp(S) is computed by ones-matmul chains directly on the 16 raw
exp tiles (PE filler work that is always ready right after exp), replacing
the baseline's DVE add-tree which serialized against AV.  All transposes run
in f32 through [P,512] psum slices so scores(4) + av(2) + z/transpose(2)
psum tags fit the 8 banks exactly.  Matmuls fp16, stage-2 mostly fp16,
sigmoid via the Exp table (no ACT table swap), y returned fp16.
"""

import sys

import numpy as np

for _p in ("/opt/trn_rl_repo", "/root/.axon_site/_ro/trn_rl_repo"):
    if _p not in sys.path:
        sys.path.append(_p)

import concourse.bass as bass  # noqa: F401
import concourse.mybir as mybir
import concourse.tile as tile
from concourse import bacc
from concourse.bass_utils import run_bass_kernel_spmd
from concourse.masks import make_identity

S, R, DH = 8, 2, 64
B, N, DIM = 2, 2048, 512
P = 128
NT = N // P  # 16 n-tiles
KC = DIM // P  # 4 contraction chunks of x
NCORES = 8
HPC = 2  # heads per core

F32 = mybir.dt.float32
F16 = mybir.dt.float16
AF = mybir.ActivationFunctionType
ALU = mybir.AluOpType


def _emit(tc, xt, wq, wk, wv, wqr, wkt, wo, y, zscr):
    from contextlib import ExitStack

    nc = tc.nc
    with ExitStack() as ctx:
        cpool = ctx.enter_context(tc.tile_pool(name="const", bufs=1))
        main = ctx.enter_context(tc.tile_pool(name="main", bufs=1))
        hd = ctx.enter_context(tc.tile_pool(name="hd", bufs=2))
        ps = ctx.enter_context(tc.tile_pool(name="ps", bufs=2, space="PSUM"))
        stp = ctx.enter_context(tc.tile_pool(name="stp", bufs=2, space="PSUM"))
        zp = ctx.enter_context(tc.tile_pool(name="zp", bufs=2, space="PSUM"))

        # ---- constants (no DMA issue yet: xT chunks go first) ----
        ident = cpool.tile([P, P], F32, name="ident")
        make_identity(nc, ident)
        ones16 = cpool.tile([P, 1], F16, name="ones16")
        nc.gpsimd.memset(ones16, 1.0)

        wq_sb = cpool.tile([P, KC, P], F16, name="wq_sb")
        wk_sb = cpool.tile([P, KC, P], F16, name="wk_sb")
        wqr_sb = cpool.tile([P, KC, P], F16, name="wqr_sb")
        wv_sb = cpool.tile([P, KC, P], F16, name="wv_sb")
        wkt_sb = cpool.tile([2 * DH, DH], F16, name="wkt_sb")
        wo_sb = cpool.tile([P, DIM], F16, name="wo_sb")

        # ---- persistent SBUF ----
        sqT = main.tile([P, N], F16, name="sqT")
        skT = main.tile([P, N], F16, name="skT")
        rqT = main.tile([P, N], F16, name="rqT")
        rv16 = main.tile([P, NT, P], F16, name="rv16")
        ET0 = main.tile([P, NT, N], F16, name="ET0")
        ufin = main.tile([P, N], F16, name="ufin")
        zrow = main.tile([1, N], F32, name="zrow")
        prod = main.tile([P, NT, R, DH], F16, name="prod")
        ybuf = main.tile([P, 4, DIM], F16, name="ybuf")
        ufin_v = ufin.rearrange("p (f c) -> p f c", c=P)
        y_v = y.rearrange("(f p) m -> p f m", p=P)

        def htile(shape, dt, tag):
            return hd.tile(shape, dt, tag=tag, name=tag)

        # ---------------- emission helpers ----------------
        def proj_chain(wsb, dst, ic):
            """One 512-col chunk of a projection: 4 chained kc matmuls."""
            pp = ps.tile([P, 512], F32, tag="av", name="pp")
            for kc in range(KC):
                nc.tensor.matmul(
                    pp,
                    wsb[:, kc, :],
                    xT[:, kc, ic * 512 : (ic + 1) * 512],
                    start=(kc == 0),
                    stop=(kc == KC - 1),
                    skip_group_check=True,
                )
            nc.vector.tensor_copy(out=dst[:, ic * 512 : (ic + 1) * 512], in_=pp)

        def rv_chunk(ic):
            """rv for one 512-col chunk: proj chain + 4 f32 transposes."""
            pv = ps.tile([P, 512], F32, tag="av", name="pv")
            for kc in range(KC):
                nc.tensor.matmul(
                    pv,
                    wv_sb[:, kc, :],
                    xT[:, kc, ic * 512 : (ic + 1) * 512],
                    start=(kc == 0),
                    stop=(kc == KC - 1),
                    skip_group_check=True,
                )
            rvT_c = hd.tile([P, 512], F32, tag="rvT", name="rvT", bufs=1)
            nc.vector.tensor_copy(out=rvT_c, in_=pv)
            pt = zp.tile([P, 4, P], F32, tag="z", name="pt")
            for t in range(4):
                nc.tensor.transpose(pt[:, t, :], rvT_c[:, t * P : (t + 1) * P], ident)
            nc.vector.tensor_copy(out=rv16[:, ic * 4 : ic * 4 + 4], in_=pt)

        def score_unit(h, ET, jt, icc):
            """Scores for (jt, icc): 2 matmuls into [P,1024] psum + exp."""
            hs = slice(h * DH, (h + 1) * DH)
            st = stp.tile([P, 1024], F32, tag="st", name="st")
            for half in range(2):
                i0 = icc * 1024 + half * 512
                nc.tensor.matmul(
                    st[:, half * 512 : (half + 1) * 512],
                    skT[hs, jt * P : (jt + 1) * P],
                    sqT[hs, i0 : i0 + 512],
                    start=True,
                    stop=True,
                )
            nc.scalar.activation(ET[:, jt, icc * 1024 : (icc + 1) * 1024], st, AF.Exp)

        def z_chain(h, ET, ic):
            """Z for one 512-col i-chunk: 16 chained ones-matmuls.
            After each 1024-col half lands, ship it to zscr (one DMA)."""
            pz = zp.tile([P, 512], F32, tag="z", name="pz")
            for jt in range(NT):
                nc.tensor.matmul(
                    pz[0:1, :],
                    ones16,
                    ET[:, jt, ic * 512 : (ic + 1) * 512],
                    start=(jt == 0),
                    stop=(jt == NT - 1),
                    skip_group_check=True,
                )
            nc.vector.tensor_copy(out=zrow[:, ic * 512 : (ic + 1) * 512], in_=pz[0:1, :])
            if ic % 2 == 1:
                h0c = (ic - 1) * 512
                nc.sync.dma_start(
                    zscr[h : h + 1, h0c : h0c + 1024], zrow[:, h0c : h0c + 1024]
                )

        def av_chain(ET, uT32, ic):
            """AV for one 512-col i-chunk: 16 chained matmuls + psum copy."""
            pa = ps.tile([P, 512], F32, tag="av", name="pa")
            for jt in range(NT):
                nc.tensor.matmul(
                    pa,
                    rv16[:, jt, :],
                    ET[:, jt, ic * 512 : (ic + 1) * 512],
                    start=(jt == 0),
                    stop=(jt == NT - 1),
                    skip_group_check=True,
                )
            nc.vector.tensor_copy(out=uT32[:, ic * 512 : (ic + 1) * 512], in_=pa)

        def u_transpose_group(uT32, un, it0):
            """Transpose u to natural layout for 4 i-tiles; one batched copy."""
            pt = zp.tile([P, 4, P], F32, tag="z", name="ut")
            for t in range(4):
                it = it0 + t
                nc.tensor.transpose(pt[:, t, :], uT32[:, it * P : (it + 1) * P], ident)
            nc.vector.tensor_copy(out=un[:, it0 : it0 + 4], in_=pt)

        def rqw_group(h, rqw, it0):
            """rqW for 8 i-tiles; one batched copy."""
            hs = slice(h * DH, (h + 1) * DH)
            pq = zp.tile([P, 8, DH], F32, tag="z", name="pq")
            for t in range(8):
                it = it0 + t
                nc.tensor.matmul(
                    pq[:, t, :],
                    rqT[hs, it * P : (it + 1) * P],
                    wkt_sb[hs, :],
                    start=True,
                    stop=True,
                )
            nc.vector.tensor_copy(out=rqw[:, it0 : it0 + 8], in_=pq)

        def zgather(h, zcol, zinv, half):
            c0 = half * 8
            nc.sync.dma_start(
                zcol[:, c0 : c0 + 8],
                zscr.rearrange("b (it p) -> b p it", p=P)[h][:, c0 : c0 + 8],
            )
            nc.vector.reciprocal(zinv[:, c0 : c0 + 8], zcol[:, c0 : c0 + 8])

        def stage2_half(h, un, rqw, uc32, zinv, sims, gd, egd, attn, aa, it0):
            """Retrieval gating for 8 i-tiles (softmax over R=2 via exp)."""
            s_ = slice(it0, it0 + 8)
            nc.vector.tensor_tensor(
                prod[:, s_],
                un[:, s_].rearrange("p it (r d) -> p it r d", r=R),
                rqw[:, s_, None, :].to_broadcast((P, 8, R, DH)),
                ALU.mult,
            )
            nc.vector.tensor_reduce(
                sims[:, s_], prod[:, s_], axis=mybir.AxisListType.X, op=ALU.add
            )
            nc.vector.tensor_tensor(
                gd[:, s_], sims[:, s_, 0], sims[:, s_, 1], ALU.subtract
            )
            nc.vector.tensor_tensor(gd[:, s_], gd[:, s_], zinv[:, s_], ALU.mult)
            # sigmoid(gd) = 1/(1+exp(-gd)) on the already-loaded Exp table
            nc.scalar.activation(egd[:, s_], gd[:, s_], AF.Exp, scale=-1.0)
            nc.vector.tensor_scalar_add(egd[:, s_], egd[:, s_], 1.0)
            nc.vector.reciprocal(attn[:, s_], egd[:, s_])
            nc.vector.tensor_tensor(aa[:, s_], attn[:, s_], zinv[:, s_], ALU.mult)
            u0 = un[:, s_, 0:DH]
            u1 = un[:, s_, DH : 2 * DH]
            nc.vector.tensor_tensor(u0, u0, u1, ALU.subtract)
            nc.vector.tensor_tensor(
                u1, u1, zinv[:, s_, None].to_broadcast((P, 8, DH)), ALU.mult
            )
            nc.vector.tensor_tensor(
                u0, u0, aa[:, s_, None].to_broadcast((P, 8, DH)), ALU.mult
            )
            nc.vector.tensor_tensor(uc32[:, s_], u0, u1, ALU.add)

        def s2tiles():
            zcol = htile([P, NT], F32, "zcol")
            zinv = htile([P, NT], F32, "zinv")
            sims = htile([P, NT, R], F32, "sims")
            gd = htile([P, NT], F32, "gd")
            egd = htile([P, NT], F32, "egd")
            attn = htile([P, NT], F32, "attn")
            aa = htile([P, NT], F32, "aa")
            return zcol, zinv, sims, gd, egd, attn, aa

        def uf_batch(h, uc32, it0):
            """Combined head output back to [d, i] for 4 i-tiles."""
            pf = zp.tile([P, 4, P], F32, tag="z", name="pf")
            for t in range(4):
                nc.tensor.transpose(pf[0:DH, t, :], uc32[:, it0 + t, :], ident)
            nc.vector.tensor_copy(
                out=ufin_v[h * DH : (h + 1) * DH, it0 : it0 + 4], in_=pf[0:DH]
            )

        def outproj(it):
            py = ps.tile([P, 512], F32, tag="av", name="py")
            nc.tensor.matmul(
                py, ufin[:, it * P : (it + 1) * P], wo_sb, start=True, stop=True
            )
            nc.vector.tensor_copy(out=ybuf[:, it % 4], in_=py)
            if it % 4 == 3:
                nc.sync.dma_start(y_v[:, it - 3 : it + 1, :], ybuf)

        # ---------------- phases A + B (xT live) ----------------
        with tc.tile_pool(name="xs", bufs=1) as xs:
            xT = xs.tile([P, KC, N], F16, name="xT")
            # DMA issue order tuned for fast start: wq + xT ic0 first so the
            # first projection chain can begin ~3us in.
            nc.sync.dma_start(wq_sb, wq.rearrange("p (kc m) -> p kc m", kc=KC))
            for kc in range(KC):
                nc.sync.dma_start(
                    xT[:, kc, 0:512], xt[kc * P : (kc + 1) * P, 0:512]
                )
            nc.sync.dma_start(wk_sb, wk.rearrange("p (kc m) -> p kc m", kc=KC))
            for ic in range(1, 4):
                for kc in range(KC):
                    nc.sync.dma_start(
                        xT[:, kc, ic * 512 : (ic + 1) * 512],
                        xt[kc * P : (kc + 1) * P, ic * 512 : (ic + 1) * 512],
                    )
            nc.sync.dma_start(wqr_sb, wqr.rearrange("p (kc m) -> p kc m", kc=KC))
            nc.sync.dma_start(wv_sb, wv.rearrange("p (kc m) -> p kc m", kc=KC))
            nc.sync.dma_start(wkt_sb[0:DH, :], wkt[:, :])
            nc.sync.dma_start(wkt_sb[DH : 2 * DH, :], wkt[:, :])
            nc.sync.dma_start(wo_sb, wo[:, :])

            # A: proj q, k
            for ic in range(4):
                proj_chain(wq_sb, sqT, ic)
                proj_chain(wk_sb, skT, ic)

            # B: scores_h0 interleaved with rq/rv fills
            uT32_0 = htile([P, N], F32, "uT32")
            fills = []
            for ic in range(4):
                fills.append(lambda ic=ic: proj_chain(wqr_sb, rqT, ic))
                fills.append(lambda ic=ic: rv_chunk(ic))
            u = 0
            for icc in range(2):
                for jt in range(NT):
                    score_unit(0, ET0, jt, icc)
                    u += 1
                    if u % 4 == 0 and fills:
                        fills.pop(0)()
            # B-end: first half of Z_h0 / AV_h0 (exp icc=0 tiles ready)
            z_chain(0, ET0, 0)
            av_chain(ET0, uT32_0, 0)
            z_chain(0, ET0, 1)
            av_chain(ET0, uT32_0, 1)

        # xT freed; ET1 reuses its SBUF range
        et1 = ctx.enter_context(tc.tile_pool(name="et1", bufs=1))
        ET1 = et1.tile([P, NT, N], F16, name="ET1")

        # ---------------- phase C ----------------
        un0 = htile([P, NT, P], F16, "un")
        rqw0 = htile([P, NT, DH], F16, "rqw")
        fills = [
            lambda: z_chain(0, ET0, 2),
            lambda: av_chain(ET0, uT32_0, 2),
            lambda: z_chain(0, ET0, 3),
            lambda: av_chain(ET0, uT32_0, 3),
            lambda: u_transpose_group(uT32_0, un0, 0),
            lambda: rqw_group(0, rqw0, 0),
            lambda: u_transpose_group(uT32_0, un0, 4),
            lambda: u_transpose_group(uT32_0, un0, 8),
            lambda: rqw_group(0, rqw0, 8),
            lambda: u_transpose_group(uT32_0, un0, 12),
        ]
        u = 0
        for icc in range(2):
            for jt in range(NT):
                score_unit(1, ET1, jt, icc)
                u += 1
                if u % 3 == 0 and fills:
                    fills.pop(0)()
        while fills:
            fills.pop(0)()
        zc0, zi0, sm0, gd0, eg0, at0, aa0 = s2tiles()
        zgather(0, zc0, zi0, 0)
        zgather(0, zc0, zi0, 1)
        uc32_0 = hd.tile([P, NT, DH], F32, tag="uc32", name="uc32", bufs=1)
        stage2_half(0, un0, rqw0, uc32_0, zi0, sm0, gd0, eg0, at0, aa0, 0)
        stage2_half(0, un0, rqw0, uc32_0, zi0, sm0, gd0, eg0, at0, aa0, 8)

        # ---------------- phase D (h1 reductions + h0 wrap-up) ----------------
        uT32_1 = htile([P, N], F32, "uT32")
        un1 = htile([P, NT, P], F16, "un")
        rqw1 = htile([P, NT, DH], F16, "rqw")
        zc1, zi1, sm1, gd1, eg1, at1, aa1 = s2tiles()

        z_chain(1, ET1, 0)
        av_chain(ET1, uT32_1, 0)
        uf_batch(0, uc32_0, 0)
        z_chain(1, ET1, 1)
        av_chain(ET1, uT32_1, 1)
        uf_batch(0, uc32_0, 4)
        u_transpose_group(uT32_1, un1, 0)
        rqw_group(1, rqw1, 0)
        u_transpose_group(uT32_1, un1, 4)
        zgather(1, zc1, zi1, 0)
        uf_batch(0, uc32_0, 8)
        uf_batch(0, uc32_0, 12)
        # all uc32 v1 (h0) reads are now emitted; stage-2 h1 may reuse it.
        # Its first half runs on the DVE while the PE continues AV/Z ic2-3.
        uc32_1 = hd.tile([P, NT, DH], F32, tag="uc32", name="uc32", bufs=1)
        stage2_half(1, un1, rqw1, uc32_1, zi1, sm1, gd1, eg1, at1, aa1, 0)
        z_chain(1, ET1, 2)
        av_chain(ET1, uT32_1, 2)
        uf_batch(1, uc32_1, 0)
        z_chain(1, ET1, 3)
        av_chain(ET1, uT32_1, 3)
        uf_batch(1, uc32_1, 4)
        u_transpose_group(uT32_1, un1, 8)
        rqw_group(1, rqw1, 8)
        u_transpose_group(uT32_1, un1, 12)
        zgather(1, zc1, zi1, 1)
        stage2_half(1, un1, rqw1, uc32_1, zi1, sm1, gd1, eg1, at1, aa1, 8)

        # ---------------- phase E ----------------
        # outproj 0-7 runs on the PE while stage-2 h1's second half drains
        # through the DVE; 8-15 follows right behind its uf transposes.
        for it in range(8):
            outproj(it)
        uf_batch(1, uc32_1, 8)
        uf_batch(1, uc32_1, 12)
        for it in range(8, NT):
            outproj(it)


def build_program():
    nc = bacc.Bacc(None, target_bir_lowering=False)
    xt = nc.declare_dram_parameter("xt", [DIM, N], F16, isOutput=False)
    wq = nc.declare_dram_parameter("wq", [P, KC * P], F16, isOutput=False)
    wk = nc.declare_dram_parameter("wk", [P, KC * P], F16, isOutput=False)
    wv = nc.declare_dram_parameter("wv", [P, KC * P], F16, isOutput=False)
    wqr = nc.declare_dram_parameter("wqr", [P, KC * P], F16, isOutput=False)
    wkt = nc.declare_dram_parameter("wkt", [DH, DH], F16, isOutput=False)
    wo = nc.declare_dram_parameter("wo", [P, DIM], F16, isOutput=False)
    y = nc.declare_dram_parameter("y", [N, DIM], F16, isOutput=True)
    zscr = nc.dram_tensor("zscr", [HPC, N], F32)

    with tile.TileContext(nc) as tc:
        _emit(tc, xt, wq, wk, wv, wqr, wkt, wo, y, zscr)
    nc.compile()
    return nc


_NC_CACHE = None


def _get_program():
    global _NC_CACHE
    if _NC_CACHE is None:
        _NC_CACHE = build_program()
    return _NC_CACHE


def make_in_maps(inputs):
    x = np.asarray(inputs["x"], dtype=np.float32)
    wq_s = np.asarray(inputs["wq_s"], dtype=np.float32)
    wk_s = np.asarray(inputs["wk_s"], dtype=np.float32)
    wv_r = np.asarray(inputs["wv_r"], dtype=np.float32)
    wq_r = np.asarray(inputs["wq_r"], dtype=np.float32)
    wk_ret = np.asarray(inputs["wk_ret"], dtype=np.float32)
    w_out = np.asarray(inputs["w_out"], dtype=np.float32)
    scale = np.float32(DH**-0.5)

    f16 = np.float16

    def _wlayout(w):
        # (kc p) x m  ->  p x (kc m): straight 1KB-per-partition DMA rows
        return np.ascontiguousarray(
            w.reshape(KC, P, P).transpose(1, 0, 2).reshape(P, KC * P)
        ).astype(f16)

    in_maps = []
    for c in range(NCORES):
        b, hp = divmod(c, NCORES // B)
        cols = slice(hp * P, (hp + 1) * P)
        in_maps.append(
            {
                "xt": np.ascontiguousarray(x[b].T).astype(f16),
                "wq": _wlayout(wq_s[:, cols] * scale),
                "wk": _wlayout(wk_s[:, cols]),
                "wv": _wlayout(wv_r),
                "wqr": _wlayout(wq_r[:, cols] * scale),
                "wkt": np.ascontiguousarray(wk_ret.T).astype(f16),
                "wo": np.ascontiguousarray(w_out[hp * P : (hp + 1) * P, :]).astype(f16),
            }
        )
    return in_maps


def run(inputs, trace=False, **kw):
    res = run_bass_kernel_spmd(
        _get_program(), make_in_maps(inputs), list(range(NCORES)), trace=trace, **kw
    )
    out = np.zeros((B, N, DIM), np.float32)
    for c in range(NCORES):
        out[c // (NCORES // B)] += np.asarray(res.results[c]["y"], np.float32)
    return out, res


def kernel(**inputs):
    out, _ = run(inputs)
    return out


# revision 10
# speedup vs baseline: 1.0244x; 1.0244x over previous
"""Compositional attention kernel for Trainium2, 8-core SPMD.

Sharding: core c handles batch b = c // 4 and search-heads {2*(c%4), 2*(c%4)+1}
end-to-end (tensor-parallel over the S=8 search heads x data-parallel over
B=2).  Each core computes a partial y = out_heads @ w_out[head_rows] for its
batch; the host sums the 4 partials per batch.

Schedule: one globally interleaved emission keeps the PE saturated (and at
full pstate clock) while the Scalar engine's exp stream - the 67us floor -
runs continuously:
  A: proj q,k (both heads packed [128, n])
  B: scores_h0 units (st psum -> exp on ACT) interleaved with proj rq, rv
     (+ PE transposes), then Z_h0/AV_h0 chains for the first half of i
  C: scores_h1 units interleaved with the rest of Z_h0/AV_h0, u_h0
     transposes, rqW_h0, stage-2 h0 (DVE)
  D: Z_h1 + AV_h1 chains interleaved, uf_h0 transposes
  E: u_h1 transposes, rqW_h1, stage-2 h1, uf_h1 + out-proj per i-tile

Z = sum_j exp(S) is computed by ones-matmul chains directly on the 16 raw
exp tiles (PE filler work that is always ready right after exp), replacing
the baseline's DVE add-tree which serialized against AV.  All transposes run
in f32 through [P,512] psum slices so scores(4) + av(2) + z/transpose(2)
psum tags fit the 8 banks exactly.  Matmuls fp16, stage-2 mostly fp16,
sigmoid via the Exp table (no ACT table swap), y returned fp16.
"""

import sys

import numpy as np

for _p in ("/opt/trn_rl_repo", "/root/.axon_site/_ro/trn_rl_repo"):
    if _p not in sys.path:
        sys.path.append(_p)

import concourse.bass as bass  # noqa: F401
import concourse.mybir as mybir
import concourse.tile as tile
from concourse import bacc
from concourse.bass_utils import run_bass_kernel_spmd
from concourse.masks import make_identity

S, R, DH = 8, 2, 64
B, N, DIM = 2, 2048, 512
P = 128
NT = N // P  # 16 n-tiles
KC = DIM // P  # 4 contraction chunks of x
NCORES = 8
HPC = 2  # heads per core

F32 = mybir.dt.float32
F16 = mybir.dt.float16
AF = mybir.ActivationFunctionType
ALU = mybir.AluOpType


def _emit(tc, xt, wq, wk, wv, wqr, wkt, wo, y, zscr):
    from contextlib import ExitStack

    nc = tc.nc
    with ExitStack() as ctx:
        cpool = ctx.enter_context(tc.tile_pool(name="const", bufs=1))
        main = ctx.enter_context(tc.tile_pool(name="main", bufs=1))
        hd = ctx.enter_context(tc.tile_pool(name="hd", bufs=2))
        ps = ctx.enter_context(tc.tile_pool(name="ps", bufs=2, space="PSUM"))
        stp = ctx.enter_context(tc.tile_pool(name="stp", bufs=2, space="PSUM"))
        zp = ctx.enter_context(tc.tile_pool(name="zp", bufs=2, space="PSUM"))

        # ---- constants (no DMA issue yet: xT chunks go first) ----
        ident = cpool.tile([P, P], F32, name="ident")
        make_identity(nc, ident)
        ones16 = cpool.tile([P, 1], F16, name="ones16")
        nc.gpsimd.memset(ones16, 1.0)

        wq_sb = cpool.tile([P, KC, P], F16, name="wq_sb")
        wk_sb = cpool.tile([P, KC, P], F16, name="wk_sb")
        wqr_sb = cpool.tile([P, KC, P], F16, name="wqr_sb")
        wv_sb = cpool.tile([P, KC, P], F16, name="wv_sb")
        wkt_sb = cpool.tile([2 * DH, DH], F16, name="wkt_sb")
        wo_sb = cpool.tile([P, DIM], F16, name="wo_sb")

        # ---- persistent SBUF ----
        sqT = main.tile([P, N], F16, name="sqT")
        skT = main.tile([P, N], F16, name="skT")
        rqT = main.tile([P, N], F16, name="rqT")
        rv16 = main.tile([P, NT, P], F16, name="rv16")
        ET0 = main.tile([P, NT, N], F16, name="ET0")
        ufin = main.tile([P, N], F16, name="ufin")
        zrow = main.tile([1, N], F32, name="zrow")
        prod = main.tile([P, NT, R, DH], F16, name="prod")
        ybuf = main.tile([P, 4, DIM], F16, name="ybuf")
        ufin_v = ufin.rearrange("p (f c) -> p f c", c=P)
        y_v = y.rearrange("(f p) m -> p f m", p=P)

        def htile(shape, dt, tag):
            return hd.tile(shape, dt, tag=tag, name=tag)

        # ---------------- emission helpers ----------------
        def proj_chain(wsb, dst, ic):
            """One 512-col chunk of a projection: 4 chained kc matmuls."""
            pp = ps.tile([P, 512], F32, tag="av", name="pp")
            for kc in range(KC):
                nc.tensor.matmul(
                    pp,
                    wsb[:, kc, :],
                    xT[:, kc, ic * 512 : (ic + 1) * 512],
                    start=(kc == 0),
                    stop=(kc == KC - 1),
                    skip_group_check=True,
                )
            nc.vector.tensor_copy(out=dst[:, ic * 512 : (ic + 1) * 512], in_=pp)

        def rv_chunk(ic):
            """rv for one 512-col chunk: proj chain + 4 f32 transposes."""
            pv = ps.tile([P, 512], F32, tag="av", name="pv")
            for kc in range(KC):
                nc.tensor.matmul(
                    pv,
                    wv_sb[:, kc, :],
                    xT[:, kc, ic * 512 : (ic + 1) * 512],
                    start=(kc == 0),
                    stop=(kc == KC - 1),
                    skip_group_check=True,
                )
            rvT_c = hd.tile([P, 512], F32, tag="rvT", name="rvT", bufs=1)
            nc.vector.tensor_copy(out=rvT_c, in_=pv)
            pt = zp.tile([P, 4, P], F32, tag="z", name="pt")
            for t in range(4):
                nc.tensor.transpose(pt[:, t, :], rvT_c[:, t * P : (t + 1) * P], ident)
            nc.vector.tensor_copy(out=rv16[:, ic * 4 : ic * 4 + 4], in_=pt)

        def score_unit(h, ET, jt, icc):
            """Scores for (jt, icc): 2 matmuls into [P,1024] psum + exp."""
            hs = slice(h * DH, (h + 1) * DH)
            st = stp.tile([P, 1024], F32, tag="st", name="st")
            for half in range(2):
                i0 = icc * 1024 + half * 512
                nc.tensor.matmul(
                    st[:, half * 512 : (half + 1) * 512],
                    skT[hs, jt * P : (jt + 1) * P],
                    sqT[hs, i0 : i0 + 512],
                    start=True,
                    stop=True,
                )
            nc.scalar.activation(ET[:, jt, icc * 1024 : (icc + 1) * 1024], st, AF.Exp)

        def z_chain(h, ET, ic):
            """Z for one 512-col i-chunk: 16 chained ones-matmuls.
            After each 1024-col half lands, ship it to zscr (one DMA)."""
            pz = zp.tile([P, 512], F32, tag="z", name="pz")
            for jt in range(NT):
                nc.tensor.matmul(
                    pz[0:1, :],
                    ones16,
                    ET[:, jt, ic * 512 : (ic + 1) * 512],
                    start=(jt == 0),
                    stop=(jt == NT - 1),
                    skip_group_check=True,
                )
            nc.vector.tensor_copy(out=zrow[:, ic * 512 : (ic + 1) * 512], in_=pz[0:1, :])
            if ic % 2 == 1:
                h0c = (ic - 1) * 512
                nc.sync.dma_start(
                    zscr[h : h + 1, h0c : h0c + 1024], zrow[:, h0c : h0c + 1024]
                )

        def av_chain(ET, uT32, ic):
            """AV for one 512-col i-chunk: 16 chained matmuls + psum copy."""
            pa = ps.tile([P, 512], F32, tag="av", name="pa")
            for jt in range(NT):
                nc.tensor.matmul(
                    pa,
                    rv16[:, jt, :],
                    ET[:, jt, ic * 512 : (ic + 1) * 512],
                    start=(jt == 0),
                    stop=(jt == NT - 1),
                    skip_group_check=True,
                )
            nc.vector.tensor_copy(out=uT32[:, ic * 512 : (ic + 1) * 512], in_=pa)

        def u_transpose_group(uT32, un, it0):
            """Transpose u to natural layout for 4 i-tiles; one batched copy."""
            pt = zp.tile([P, 4, P], F32, tag="z", name="ut")
            for t in range(4):
                it = it0 + t
                nc.tensor.transpose(pt[:, t, :], uT32[:, it * P : (it + 1) * P], ident)
            nc.vector.tensor_copy(out=un[:, it0 : it0 + 4], in_=pt)

        def rqw_group(h, rqw, it0):
            """rqW for 8 i-tiles; one batched copy."""
            hs = slice(h * DH, (h + 1) * DH)
            pq = zp.tile([P, 8, DH], F32, tag="z", name="pq")
            for t in range(8):
                it = it0 + t
                nc.tensor.matmul(
                    pq[:, t, :],
                    rqT[hs, it * P : (it + 1) * P],
                    wkt_sb[hs, :],
                    start=True,
                    stop=True,
                )
            nc.vector.tensor_copy(out=rqw[:, it0 : it0 + 8], in_=pq)

        def zgather(h, zcol, zinv, half):
            c0 = half * 8
            nc.sync.dma_start(
                zcol[:, c0 : c0 + 8],
                zscr.rearrange("b (it p) -> b p it", p=P)[h][:, c0 : c0 + 8],
            )
            nc.vector.reciprocal(zinv[:, c0 : c0 + 8], zcol[:, c0 : c0 + 8])

        def stage2_half(h, un, rqw, uc32, zinv, sims, gd, egd, attn, aa, it0):
            """Retrieval gating for 8 i-tiles (softmax over R=2 via exp)."""
            s_ = slice(it0, it0 + 8)
            nc.vector.tensor_tensor(
                prod[:, s_],
                un[:, s_].rearrange("p it (r d) -> p it r d", r=R),
                rqw[:, s_, None, :].to_broadcast((P, 8, R, DH)),
                ALU.mult,
            )
            nc.vector.tensor_reduce(
                sims[:, s_], prod[:, s_], axis=mybir.AxisListType.X, op=ALU.add
            )
            nc.vector.tensor_tensor(
                gd[:, s_], sims[:, s_, 0], sims[:, s_, 1], ALU.subtract
            )
            nc.vector.tensor_tensor(gd[:, s_], gd[:, s_], zinv[:, s_], ALU.mult)
            # sigmoid(gd) = 1/(1+exp(-gd)) on the already-loaded Exp table
            nc.scalar.activation(egd[:, s_], gd[:, s_], AF.Exp, scale=-1.0)
            nc.vector.tensor_scalar_add(egd[:, s_], egd[:, s_], 1.0)
            nc.vector.reciprocal(attn[:, s_], egd[:, s_])
            nc.vector.tensor_tensor(aa[:, s_], attn[:, s_], zinv[:, s_], ALU.mult)
            u0 = un[:, s_, 0:DH]
            u1 = un[:, s_, DH : 2 * DH]
            nc.vector.tensor_tensor(u0, u0, u1, ALU.subtract)
            nc.vector.tensor_tensor(
                u1, u1, zinv[:, s_, None].to_broadcast((P, 8, DH)), ALU.mult
            )
            nc.vector.tensor_tensor(
                u0, u0, aa[:, s_, None].to_broadcast((P, 8, DH)), ALU.mult
            )
            nc.vector.tensor_tensor(uc32[:, s_], u0, u1, ALU.add)

        def s2tiles():
            zcol = htile([P, NT], F32, "zcol")
            zinv = htile([P, NT], F32, "zinv")
            sims = htile([P, NT, R], F32, "sims")
            gd = htile([P, NT], F32, "gd")
            egd = htile([P, NT], F32, "egd")
            attn = htile([P, NT], F32, "attn")
            aa = htile([P, NT], F32, "aa")
            return zcol, zinv, sims, gd, egd, attn, aa

        def uf_batch(h, uc32, it0):
            """Combined head output back to [d, i] for 4 i-tiles."""
            pf = zp.tile([P, 4, P], F32, tag="z", name="pf")
            for t in range(4):
                nc.tensor.transpose(pf[0:DH, t, :], uc32[:, it0 + t, :], ident)
            nc.vector.tensor_copy(
                out=ufin_v[h * DH : (h + 1) * DH, it0 : it0 + 4], in_=pf[0:DH]
            )

        def outproj(it):
            py = ps.tile([P, 512], F32, tag="av", name="py")
            nc.tensor.matmul(
                py, ufin[:, it * P : (it + 1) * P], wo_sb, start=True, stop=True
            )
            nc.vector.tensor_copy(out=ybuf[:, it % 4], in_=py)
            if it % 4 == 3:
                nc.sync.dma_start(y_v[:, it - 3 : it + 1, :], ybuf)

        # ---------------- phases A + B (xT live) ----------------
        with tc.tile_pool(name="xs", bufs=1) as xs:
            xT = xs.tile([P, KC, N], F16, name="xT")
            # DMA issue order tuned for fast start: wq + xT ic0 first so the
            # first projection chain can begin ~3us in.
            nc.sync.dma_start(wq_sb, wq.rearrange("p (kc m) -> p kc m", kc=KC))
            for kc in range(KC):
                nc.sync.dma_start(
                    xT[:, kc, 0:512], xt[kc * P : (kc + 1) * P, 0:512]
                )
            nc.sync.dma_start(wk_sb, wk.rearrange("p (kc m) -> p kc m", kc=KC))
            for ic in range(1, 4):
                for kc in range(KC):
                    nc.sync.dma_start(
                        xT[:, kc, ic * 512 : (ic + 1) * 512],
                        xt[kc * P : (kc + 1) * P, ic * 512 : (ic + 1) * 512],
                    )
            nc.sync.dma_start(wqr_sb, wqr.rearrange("p (kc m) -> p kc m", kc=KC))
            nc.sync.dma_start(wv_sb, wv.rearrange("p (kc m) -> p kc m", kc=KC))
            nc.sync.dma_start(wkt_sb[0:DH, :], wkt[:, :])
            nc.sync.dma_start(wkt_sb[DH : 2 * DH, :], wkt[:, :])
            nc.sync.dma_start(wo_sb, wo[:, :])

            # A: proj q, k
            for ic in range(4):
                proj_chain(wq_sb, sqT, ic)
                proj_chain(wk_sb, skT, ic)

            # B: scores_h0 interleaved with rq/rv fills
            uT32_0 = htile([P, N], F32, "uT32")
            fills = []
            for ic in range(4):
                fills.append(lambda ic=ic: proj_chain(wqr_sb, rqT, ic))
                fills.append(lambda ic=ic: rv_chunk(ic))
            u = 0
            for icc in range(2):
                for jt in range(NT):
                    score_unit(0, ET0, jt, icc)
                    u += 1
                    if u % 4 == 0 and fills:
                        fills.pop(0)()
            # B-end: first half of Z_h0 / AV_h0 (exp icc=0 tiles ready)
            z_chain(0, ET0, 0)
            av_chain(ET0, uT32_0, 0)
            z_chain(0, ET0, 1)
            av_chain(ET0, uT32_0, 1)

        # xT freed; ET1 reuses its SBUF range
        et1 = ctx.enter_context(tc.tile_pool(name="et1", bufs=1))
        ET1 = et1.tile([P, NT, N], F16, name="ET1")

        # ---------------- phase C ----------------
        un0 = htile([P, NT, P], F16, "un")
        rqw0 = htile([P, NT, DH], F16, "rqw")
        fills = [
            lambda: z_chain(0, ET0, 2),
            lambda: av_chain(ET0, uT32_0, 2),
            lambda: z_chain(0, ET0, 3),
            lambda: av_chain(ET0, uT32_0, 3),
            lambda: u_transpose_group(uT32_0, un0, 0),
            lambda: rqw_group(0, rqw0, 0),
            lambda: u_transpose_group(uT32_0, un0, 4),
            lambda: u_transpose_group(uT32_0, un0, 8),
            lambda: rqw_group(0, rqw0, 8),
            lambda: u_transpose_group(uT32_0, un0, 12),
        ]
        u = 0
        for icc in range(2):
            for jt in range(NT):
                score_unit(1, ET1, jt, icc)
                u += 1
                if u % 3 == 0 and fills:
                    fills.pop(0)()
        while fills:
            fills.pop(0)()
        zc0, zi0, sm0, gd0, eg0, at0, aa0 = s2tiles()
        zgather(0, zc0, zi0, 0)
        zgather(0, zc0, zi0, 1)
        uc32_0 = hd.tile([P, NT, DH], F32, tag="uc32", name="uc32", bufs=1)
        stage2_half(0, un0, rqw0, uc32_0, zi0, sm0, gd0, eg0, at0, aa0, 0)
        stage2_half(0, un0, rqw0, uc32_0, zi0, sm0, gd0, eg0, at0, aa0, 8)

        # ---------------- phase D (h1 reductions + h0 wrap-up) ----------------
        uT32_1 = htile([P, N], F32, "uT32")
        un1 = htile([P, NT, P], F16, "un")
        rqw1 = htile([P, NT, DH], F16, "rqw")
        zc1, zi1, sm1, gd1, eg1, at1, aa1 = s2tiles()

        z_chain(1, ET1, 0)
        av_chain(ET1, uT32_1, 0)
        uf_batch(0, uc32_0, 0)
        z_chain(1, ET1, 1)
        av_chain(ET1, uT32_1, 1)
        uf_batch(0, uc32_0, 4)
        u_transpose_group(uT32_1, un1, 0)
        rqw_group(1, rqw1, 0)
        u_transpose_group(uT32_1, un1, 4)
        zgather(1, zc1, zi1, 0)
        uf_batch(0, uc32_0, 8)
        uf_batch(0, uc32_0, 12)
        # all uc32 v1 (h0) reads are now emitted; stage-2 h1 may reuse it.
        # Its first half runs on the DVE while the PE continues AV/Z ic2-3.
        uc32_1 = hd.tile([P, NT, DH], F32, tag="uc32", name="uc32", bufs=1)
        stage2_half(1, un1, rqw1, uc32_1, zi1, sm1, gd1, eg1, at1, aa1, 0)
        z_chain(1, ET1, 2)
        av_chain(ET1, uT32_1, 2)
        z_chain(1, ET1, 3)
        av_chain(ET1, uT32_1, 3)
        u_transpose_group(uT32_1, un1, 8)
        rqw_group(1, rqw1, 8)
        u_transpose_group(uT32_1, un1, 12)
        zgather(1, zc1, zi1, 1)
        uf_batch(1, uc32_1, 0)
        uf_batch(1, uc32_1, 4)
        stage2_half(1, un1, rqw1, uc32_1, zi1, sm1, gd1, eg1, at1, aa1, 8)

        # ---------------- phase E ----------------
        # outproj 0-7 runs on the PE while stage-2 h1's second half drains
        # through the DVE; 8-15 follows right behind its uf transposes.
        for it in range(8):
            outproj(it)
        uf_batch(1, uc32_1, 8)
        uf_batch(1, uc32_1, 12)
        for it in range(8, NT):
            outproj(it)


def build_program():
    nc = bacc.Bacc(None, target_bir_lowering=False)
    xt = nc.declare_dram_parameter("xt", [DIM, N], F16, isOutput=False)
    wq = nc.declare_dram_parameter("wq", [P, KC * P], F16, isOutput=False)
    wk = nc.declare_dram_parameter("wk", [P, KC * P], F16, isOutput=False)
    wv = nc.declare_dram_parameter("wv", [P, KC * P], F16, isOutput=False)
    wqr = nc.declare_dram_parameter("wqr", [P, KC * P], F16, isOutput=False)
    wkt = nc.declare_dram_parameter("wkt", [DH, DH], F16, isOutput=False)
    wo = nc.declare_dram_parameter("wo", [P, DIM], F16, isOutput=False)
    y = nc.declare_dram_parameter("y", [N, DIM], F16, isOutput=True)
    zscr = nc.dram_tensor("zscr", [HPC, N], F32)

    with tile.TileContext(nc) as tc:
        _emit(tc, xt, wq, wk, wv, wqr, wkt, wo, y, zscr)
    nc.compile()
    return nc


_NC_CACHE = None


def _get_program():
    global _NC_CACHE
    if _NC_CACHE is None:
        _NC_CACHE = build_program()
    return _NC_CACHE


def make_in_maps(inputs):
    x = np.asarray(inputs["x"], dtype=np.float32)
    wq_s = np.asarray(inputs["wq_s"], dtype=np.float32)
    wk_s = np.asarray(inputs["wk_s"], dtype=np.float32)
    wv_r = np.asarray(inputs["wv_r"], dtype=np.float32)
    wq_r = np.asarray(inputs["wq_r"], dtype=np.float32)
    wk_ret = np.asarray(inputs["wk_ret"], dtype=np.float32)
    w_out = np.asarray(inputs["w_out"], dtype=np.float32)
    scale = np.float32(DH**-0.5)

    f16 = np.float16

    def _wlayout(w):
        # (kc p) x m  ->  p x (kc m): straight 1KB-per-partition DMA rows
        return np.ascontiguousarray(
            w.reshape(KC, P, P).transpose(1, 0, 2).reshape(P, KC * P)
        ).astype(f16)

    in_maps = []
    for c in range(NCORES):
        b, hp = divmod(c, NCORES // B)
        cols = slice(hp * P, (hp + 1) * P)
        in_maps.append(
            {
                "xt": np.ascontiguousarray(x[b].T).astype(f16),
                "wq": _wlayout(wq_s[:, cols] * scale),
                "wk": _wlayout(wk_s[:, cols]),
                "wv": _wlayout(wv_r),
                "wqr": _wlayout(wq_r[:, cols] * scale),
                "wkt": np.ascontiguousarray(wk_ret.T).astype(f16),
                "wo": np.ascontiguousarray(w_out[hp * P : (hp + 1) * P, :]).astype(f16),
            }
        )
    return in_maps


def run(inputs, trace=False, **kw):
    res = run_bass_kernel_spmd(
        _get_program(), make_in_maps(inputs), list(range(NCORES)), trace=trace, **kw
    )
    out = np.zeros((B, N, DIM), np.float32)
    for c in range(NCORES):
        out[c // (NCORES // B)] += np.asarray(res.results[c]["y"], np.float32)
    return out, res


def kernel(**inputs):
    out, _ = run(inputs)
    return out
